# revision 27
# baseline (speedup 1.0000x reference)
"""Trainium2 Bass kernel for nn_AttentionModel (sparse_attention).

Reference computation:
    x = emb_table[tokens]                  # [B,S,D]
    scores = x @ x^T per batch             # [B,S,S]
    out = softmax(scores) @ x              # [B,S,D]
    logits = out[:, 0, :] @ cls_w.T + cls_b

Only row 0 of the attention output is used, so per batch element we only
need attention row 0:
    q = x[0]; s = X q; a = exp(s); logits = (a^T X / sum(a)) cls_w^T + cls_b

Sharding: data-parallel over batch. 8 cores x 4 batch elements; emb_table
and classifier weights replicated; no collectives.

Structure (per core):
  - a tiny up-front dma_gather (BPC real indices padded to 16) pulls the
    query rows so the q broadcasts never wait on the big gathers.
  - all big gathers are issued up front and run back-to-back on the DMA
    engines: per batch one bf16 token-major dma_gather (2048 x 1KB rows,
    x[t%128, t//128, :]) plus one fp8(e4m3, x16) TRANSPOSED dma_gather
    (d on partitions) covering that batch's PE-scored blocks.
  - scores s_t = <x_t, q> are split per batch across three engines
    (NPE_B / NPOOL_B / rest-on-DVE blocks of 128 tokens):
      * PE blocks: K=128 x N=1 matmul chains over the fp8 tiles; scores
        land token-on-partition in PSUM, one exp on ACT per batch.
      * Pool blocks: fused scalar_tensor_tensor + accum on GpSimd.
      * DVE blocks: one 2x-mode bf16 multiply against a stride-0 broadcast
        of q, a pairwise bf16 add tree (512->32, all 2x), f32 tensor_reduce.
    The last batch is scored fully on PE so the kernel tail never waits on
    a DVE tree.
  - finish phase (Z via ones-matmul, pooled = a^T X as a 16-matmul PSUM
    chain replicated to C partitions, psb copy, logits tensor_tensor_reduce
    against cls_w, reciprocal-scale + bias) is emitted for ALL batches
    AFTER all scoring: engines issue in-order from shallow queues, so a
    finish op stalled on a late input would otherwise block the next
    batch's scoring work behind it in its engine stream.
"""

import numpy as np

import bass_rust

import concourse.bass as bass
import concourse.mybir as mybir
import concourse.tile as tile
from concourse.bass_utils import run_bass_kernel_spmd


def _split_multiwaits(nc: bass.Bass) -> None:
    """Workaround for the walrus build in this container, which rejects
    instructions carrying more than one sync-wait command ("Too many sync
    wait commands" / "ISA wrong length" in CoreV3GenImpl setupSyncWait).

    Moves each instruction's sync waits onto dedicated single-wait NOPs
    inserted right before it on the same engine stream (bass_nofuse so
    walrus's nop-fusion can't merge them back). Semantics are identical:
    the engine's sequencer blocks on the NOPs' waits, then issues the
    now-waitless instruction. Sem updates stay on the original."""
    counter = 0
    fn = nc.m.functions[0]
    for bb in fn.blocks:
        insts = bb.instructions
        new_list = []
        changed = False
        for inst in insts:
            si = inst.sync_info
            waits = list(si.on_wait) if si is not None else []
            if waits:
                for w in waits:
                    counter += 1
                    new_list.append(
                        mybir.InstNoOp(
                            name=f"waitnop-{counter}",
                            engine=inst.engine,
                            ins=[],
                            outs=[],
                            bass_nofuse=True,
                            sync_info=bass_rust.SyncInfo(on_wait=[w], on_update=[]),
                        )
                    )
                inst.sync_info = bass_rust.SyncInfo(
                    on_wait=[], on_update=list(si.on_update)
                )
                changed = True
            new_list.append(inst)
        if changed:
            bb.instructions = new_list

B, S, D, V, C = 32, 2048, 512, 32000, 2
N_CORES = 8
BPC = B // N_CORES          # batch elements per core
SBLK = S // 16 // 8         # 16 blocks of 128 tokens

# Per-batch score-block split: [PE, Pool] (rest on DVE).  NPE even.
import os as _os
NPE_B = [int(v) for v in _os.environ.get("K_NPE", "0,0,0,0").split(",")]
NPOOL_B = [int(v) for v in _os.environ.get("K_NPOOL", "0,0,0,0").split(",")]
TREE_MIN = int(_os.environ.get("K_TREEMIN", "64"))
FP8_SCALE = 16.0            # emb values ~N(0, 0.02): x16 keeps e4m3 normal

F32 = mybir.dt.float32
BF16 = mybir.dt.bfloat16
F8 = mybir.dt.float8e4
I16 = mybir.dt.int16
I32 = mybir.dt.int32

_CACHE: dict = {}


def _build_nc(split: bool = True) -> bass.Bass:
    assert len(NPE_B) == len(NPOOL_B) == BPC
    for npe, npool in zip(NPE_B, NPOOL_B):
        assert npe % 2 == 0 and 0 <= npe + npool <= SBLK
    nc = bass.Bass()
    emb_d = nc.dram_tensor("emb", [V, D], BF16, kind="ExternalInput")
    emb8_d = (
        nc.dram_tensor("emb8", [V, D], F8, kind="ExternalInput")
        if any(NPE_B) else None
    )
    # int32 gather indices: [128, 1 + BPC*16]; col 0 rows 0..BPC-1 = token0
    # of batch b, then per batch 16 cols with idx[t%128, t//128] = token t.
    tok_d = nc.dram_tensor("tok", [128, 1 + BPC * SBLK], I32,
                           kind="ExternalInput")
    sel_d = nc.dram_tensor("sel", [BPC, BPC * 128], BF16, kind="ExternalInput")
    cw_d = nc.dram_tensor("cls_w", [C, D], BF16, kind="ExternalInput")
    cb_d = nc.dram_tensor("cls_b", [C, 1], F32, kind="ExternalInput")
    out_d = nc.dram_tensor("out", [BPC, C], F32, kind="ExternalOutput")

    mult = mybir.AluOpType.mult
    add = mybir.AluOpType.add
    EXP = mybir.ActivationFunctionType.Exp

    with tile.TileContext(nc) as tc:
        with (
            tc.tile_pool(name="const", bufs=1) as constp,
            tc.tile_pool(name="xp", bufs=1) as xp,
            tc.tile_pool(name="sp", bufs=2) as sp,
            tc.tile_pool(name="jp", bufs=2) as jp,
            tc.tile_pool(name="tp", bufs=2) as tp,
            tc.tile_pool(name="ps", bufs=2, space="PSUM") as pp,
        ):
            idx = constp.tile([128, 1 + BPC * SBLK], I32)
            nc.sync.dma_start(idx[:, :], tok_d[:, :])
            ones128 = constp.tile([128, 1], F32)
            nc.vector.memset(ones128[:], 1.0)
            # selection matrix: sel[i, b*128+m] = (i == b), so
            # sel[:, b*128:(b+1)*128]^T @ qx[0:BPC] broadcasts q_b to all
            # 128 partitions with a base-0 K=BPC matmul.
            sel = constp.tile([BPC, BPC * 128], BF16)
            nc.sync.dma_start(sel[:], sel_d[:, :])
            cw = constp.tile([C, D], BF16)
            nc.sync.dma_start(cw[:], cw_d[:, :])
            cb = constp.tile([C, 1], F32)
            nc.sync.dma_start(cb[:], cb_d[:, :])

            def bidx(b):
                return idx[:, 1 + b * SBLK : 1 + (b + 1) * SBLK]

            # --- tiny query-row gather: partition b holds q_b
            qx = constp.tile([128, D], BF16)
            nc.gpsimd.indirect_dma_start(
                out=qx[:, :],
                out_offset=None,
                in_=emb_d[:, :],
                in_offset=bass.IndirectOffsetOnAxis(ap=idx[:, 0:1], axis=0),
            )
            qbss = []
            for b in range(BPC):
                if NPE_B[b] + NPOOL_B[b] == SBLK:
                    qbss.append(None)
                    continue
                qb = pp.tile([128, D], F32, tag="qb")
                nc.tensor.matmul(
                    qb[:], sel[:, b * 128 : (b + 1) * 128], qx[0:BPC, :],
                    start=True, stop=True,
                )
                qbs = sp.tile([128, D], BF16, tag=f"qbs{b}")
                nc.scalar.copy(qbs[:], qb[:])
                qbss.append(qbs)

            # --- all big gathers up front, back-to-back on the DMA engines
            xs, xt8s = [], []
            for b in range(BPC):
                npe = NPE_B[b]
                assert npe == 0, "PE scoring needs dma_gather (unavailable)"
                xt8s.append(None)
                x = xp.tile([128, SBLK, D], BF16, tag=f"x{b}")
                for j in range(SBLK):
                    nc.gpsimd.indirect_dma_start(
                        out=x[:, j, :],
                        out_offset=None,
                        in_=emb_d[:, :],
                        in_offset=bass.IndirectOffsetOnAxis(
                            ap=bidx(b)[:, j : j + 1], axis=0
                        ),
                    )
                xs.append(x)

            state: dict[int, dict] = {}

            def score(b):
                st = {}
                npe, npool = NPE_B[b], NPOOL_B[b]
                ndve = SBLK - npe - npool
                x = xs[b]
                s = None
                if npool or SBLK - npe - npool:
                    s = sp.tile([128, SBLK], F32, tag="s")

                # PE blocks from the fp8 transposed gather.  fp8 pairs are
                # 16-bit-interleaved: element d = 2*(g2*128+p)+k of token t
                # lives at tile[p, 2*g2 + (2t+k)//NT, (2t+k) % NT].
                if npe:
                    nt = npe * 128
                    xt8 = xt8s[b]
                    sT = pp.tile([128, npe], F32, tag="sT")
                    for c in range(npe):
                        m = (256 * c) // nt
                        r0 = 256 * c - m * nt
                        combos = [(g2, k) for g2 in range(2) for k in range(2)]
                        for i, (g2, k) in enumerate(combos):
                            lhsT = xt8[:, 2 * g2 + m, r0 + k : r0 + k + 255 : 2]
                            rhs = xt8[:, 2 * g2, k : k + 1]
                            nc.tensor.matmul(
                                sT[:, c : c + 1], lhsT, rhs,
                                start=(i == 0), stop=(i == 3),
                            )
                    aT = sp.tile([128, npe], BF16, tag=f"aT{b}")
                    zc1 = sp.tile([128, 1], F32, tag=f"zc1{b}")
                    nc.scalar.activation(
                        aT[:], sT[:], EXP,
                        scale=1.0 / (FP8_SCALE * FP8_SCALE),
                        accum_out=zc1[:],
                    )
                    st["aT"], st["zc1"] = aT, zc1

                # Pool blocks: fused mult + row-reduce on GpSimd
                if npool:
                    junkP = jp.tile([128, npool, D], BF16, tag="junkP")
                    for jj in range(npool):
                        j = npe + jj
                        nc.gpsimd.scalar_tensor_tensor(
                            out=junkP[:, jj, :],
                            in0=x[:, j, :],
                            scalar=1.0,
                            in1=qbss[b][:],
                            op0=mult,
                            op1=mult,
                            accum_out=s[:, j : j + 1],
                        )

                # DVE blocks: 2x multiply + bf16 add tree + f32 reduce
                if ndve:
                    lo = npe + npool
                    jA = jp.tile([128, ndve, D], BF16, tag="jA")
                    qbc = qbss[b][:, :].unsqueeze(1).broadcast_to([128, ndve, D])
                    nc.vector.tensor_tensor(jA[:], x[:, lo:SBLK, :], qbc, op=mult)
                    w = D
                    cur = jA
                    while w > TREE_MIN:
                        w //= 2
                        nxt = jp.tile([128, ndve, w], BF16, tag=f"j{w}")
                        nc.vector.tensor_tensor(
                            nxt[:], cur[:, :, 0:w], cur[:, :, w : 2 * w], op=add
                        )
                        cur = nxt
                    nc.vector.tensor_reduce(
                        s[:, lo:SBLK], cur[:], mybir.AxisListType.X, add
                    )

                # exp for Pool+DVE cols (split so pooled never waits on both)
                zchain = [st["zc1"]] if npe else []
                if npool:
                    aP = sp.tile([128, npool], BF16, tag=f"aP{b}")
                    zcP = sp.tile([128, 1], F32, tag=f"zcP{b}")
                    nc.scalar.activation(
                        aP[:], s[:, npe : npe + npool], EXP, accum_out=zcP[:]
                    )
                    st["aP"] = aP
                    zchain.append(zcP)
                if ndve:
                    aD = sp.tile([128, ndve], BF16, tag=f"aD{b}")
                    zcD = sp.tile([128, 1], F32, tag=f"zcD{b}")
                    nc.scalar.activation(
                        aD[:], s[:, npe + npool : SBLK], EXP, accum_out=zcD[:]
                    )
                    st["aD"] = aD
                    zchain.append(zcD)
                st["zchain"] = zchain
                state[b] = st

            def finish(b):
                st = state.pop(b)
                npe, npool = NPE_B[b], NPOOL_B[b]
                x = xs[b]

                # Z = sum of all a, replicated to C partitions via M=C
                zps = pp.tile([C, 1], F32, tag="z")
                zchain = st["zchain"]
                for i, zc in enumerate(zchain):
                    nc.tensor.matmul(
                        zps[:], zc[:].broadcast_to([128, C]), ones128[:],
                        start=(i == 0), stop=(i == len(zchain) - 1),
                    )

                # pooled = a^T X, 16-matmul PSUM chain, M=C replication
                pooled = pp.tile([C, D], F32, tag="pooled")
                for j in range(SBLK):
                    if j < npe:
                        lhs = st["aT"][:, j : j + 1]
                    elif j < npe + npool:
                        lhs = st["aP"][:, j - npe : j - npe + 1]
                    else:
                        lhs = st["aD"][:, j - npe - npool : j - npe - npool + 1]
                    nc.tensor.matmul(
                        pooled[:], lhs.broadcast_to([128, C]), x[:, j, :],
                        start=(j == 0), stop=(j == SBLK - 1),
                    )

                # logits = pooled @ cls_w^T / Z + cls_b
                psb = tp.tile([C, D], BF16, tag="psb")
                nc.scalar.copy(psb[:], pooled[:])
                zsb = tp.tile([C, 1], F32, tag="zsb")
                nc.scalar.copy(zsb[:], zps[:])
                rz = tp.tile([C, 1], F32, tag="rz")
                nc.vector.reciprocal(rz[:], zsb[:])
                junkL = tp.tile([C, D], BF16, tag="junkL")
                lg = tp.tile([C, 1], F32, tag="lg")
                nc.vector.scalar_tensor_tensor(
                    out=junkL[:], in0=psb[:], scalar=1.0, in1=cw[:],
                    op0=mult, op1=mult, accum_out=lg[:],
                )
                ob = tp.tile([C, 1], F32, tag="ob")
                nc.vector.scalar_tensor_tensor(
                    out=ob[:], in0=lg[:], scalar=rz[:], in1=cb[:],
                    op0=mult, op1=add,
                )
                nc.sync.dma_start(out_d[b, :].unsqueeze(1), ob[:, :])

            score(0)
            for b in range(1, BPC):
                score(b)
                finish(b - 1)
            finish(BPC - 1)

    nc.finalize()
    if split:
        _split_multiwaits(nc)
    return nc


def _wrap_tokens(tokens_row: np.ndarray) -> np.ndarray:
    """[S] tokens -> [128, SBLK] int32; token t at [t%128, t//128]."""
    return np.ascontiguousarray(tokens_row.reshape(SBLK, 128).T.astype(np.int32))


def get_nc() -> bass.Bass:
    if "nc" not in _CACHE:
        _CACHE["nc"] = _build_nc()
    return _CACHE["nc"]


def make_in_maps(tokens, emb_table, cls_w, cls_b):
    import ml_dtypes

    tokens = np.asarray(tokens)
    emb_f32 = np.asarray(emb_table, dtype=np.float32)
    emb = np.ascontiguousarray(emb_f32.astype(ml_dtypes.bfloat16))
    if any(NPE_B) and "emb8" not in _CACHE:
        _CACHE["emb8"] = np.ascontiguousarray(
            (emb_f32 * FP8_SCALE).astype(ml_dtypes.float8_e4m3)
        )
    cw = np.ascontiguousarray(
        np.asarray(cls_w, dtype=np.float32).astype(ml_dtypes.bfloat16)
    )
    cb = np.ascontiguousarray(
        np.asarray(cls_b, dtype=np.float32).reshape(C, 1)
    )
    in_maps = []
    for core in range(N_CORES):
        tks = tokens[core * BPC : (core + 1) * BPC]
        qcol = np.zeros((128, 1), np.int32)
        qcol[:BPC, 0] = tks[:, 0].astype(np.int32)
        idx_flat = np.concatenate(
            [qcol] + [_wrap_tokens(tks[b]) for b in range(BPC)],
            axis=1,
        )  # [128, 1 + BPC * SBLK]
        sel = np.zeros((BPC, BPC * 128), np.float32)
        for b in range(BPC):
            sel[b, b * 128 : (b + 1) * 128] = 1.0
        m = {
                "emb": emb,
                "tok": np.ascontiguousarray(idx_flat),
                "sel": np.ascontiguousarray(sel.astype(ml_dtypes.bfloat16)),
                "cls_w": cw,
                "cls_b": cb,
        }
        if any(NPE_B):
            m["emb8"] = _CACHE["emb8"]
        in_maps.append(m)
    return in_maps


def kernel(tokens, emb_table, cls_w, cls_b) -> np.ndarray:
    nc = get_nc()
    in_maps = make_in_maps(tokens, emb_table, cls_w, cls_b)
    res = run_bass_kernel_spmd(nc, in_maps, core_ids=list(range(N_CORES)))
    outs = [res.results[c]["out"] for c in range(N_CORES)]
    return np.concatenate(outs, axis=0).astype(np.float32)


# revision 28
# speedup vs baseline: 1.1189x; 1.1189x over previous
"""Trainium2 Bass kernel for nn_AttentionModel (sparse_attention).

Reference computation:
    x = emb_table[tokens]                  # [B,S,D]
    scores = x @ x^T per batch             # [B,S,S]
    out = softmax(scores) @ x              # [B,S,D]
    logits = out[:, 0, :] @ cls_w.T + cls_b

Only row 0 of the attention output is used, so per batch element we only
need attention row 0:
    q = x[0]; s = X q; a = exp(s); logits = (a^T X / sum(a)) cls_w^T + cls_b
This turns ~275 GFLOP of full attention into an embedding gather plus
~4 MFLOP per batch element.

Sharding: data-parallel over batch. 8 cores x 4 batch elements; emb_table
and classifier weights replicated; no collectives.
"""

import numpy as np

import bass_rust

import concourse.bass as bass
import concourse.mybir as mybir
import concourse.tile as tile
from concourse.bass_utils import run_bass_kernel_spmd


def _split_multiwaits(nc: bass.Bass) -> None:
    """Workaround for the walrus build in this container, which rejects
    instructions carrying more than one sync-wait command ("Too many sync
    wait commands" / "ISA wrong length" in CoreV3GenImpl setupSyncWait).

    Moves each instruction's sync waits onto dedicated single-wait NOPs
    inserted right before it on the same engine stream (bass_nofuse so
    walrus's nop-fusion can't merge them back). Semantics are identical:
    the engine's sequencer blocks on the NOPs' waits, then issues the
    now-waitless instruction. Sem updates stay on the original."""
    counter = 0
    fn = nc.m.functions[0]
    for bb in fn.blocks:
        insts = bb.instructions
        new_list = []
        changed = False
        for inst in insts:
            si = inst.sync_info
            waits = list(si.on_wait) if si is not None else []
            if waits:
                for w in waits:
                    counter += 1
                    new_list.append(
                        mybir.InstNoOp(
                            name=f"waitnop-{counter}",
                            engine=inst.engine,
                            ins=[],
                            outs=[],
                            bass_nofuse=True,
                            sync_info=bass_rust.SyncInfo(on_wait=[w], on_update=[]),
                        )
                    )
                inst.sync_info = bass_rust.SyncInfo(
                    on_wait=[], on_update=list(si.on_update)
                )
                changed = True
            new_list.append(inst)
        if changed:
            bb.instructions = new_list

B, S, D, V, C = 32, 2048, 512, 32000, 2
N_CORES = 8
BPC = B // N_CORES          # batch elements per core
SBLK = S // 128             # 16 free-dim blocks of gathered tokens

F32 = mybir.dt.float32
BF16 = mybir.dt.bfloat16
I32 = mybir.dt.int32

_CACHE: dict = {}


def _build_nc(split: bool = True) -> bass.Bass:
    nc = bass.Bass()
    emb_d = nc.dram_tensor("emb", [V, D], BF16, kind="ExternalInput")
    tok_d = nc.dram_tensor("tok", [BPC, 128, SBLK], I32, kind="ExternalInput")
    cw_d = nc.dram_tensor("cls_w", [1, C * D], F32, kind="ExternalInput")
    cb_d = nc.dram_tensor("cls_b", [1, C], F32, kind="ExternalInput")
    out_d = nc.dram_tensor("out", [BPC, C], F32, kind="ExternalOutput")

    mult = mybir.AluOpType.mult
    add = mybir.AluOpType.add
    EXP = mybir.ActivationFunctionType.Exp

    with tile.TileContext(nc) as tc:
        with (
            tc.tile_pool(name="const", bufs=1) as constp,
            tc.tile_pool(name="xp", bufs=2) as xp,
            tc.tile_pool(name="sp", bufs=2) as sp,
            tc.tile_pool(name="jp", bufs=2) as jp,
            tc.tile_pool(name="tp", bufs=2) as tp,
            tc.tile_pool(name="ps", bufs=2, space="PSUM") as pp,
        ):
            ones1 = constp.tile([1, 128], BF16)
            nc.vector.memset(ones1[:], 1.0)
            ones128 = constp.tile([128, 1], F32)
            nc.vector.memset(ones128[:], 1.0)
            cw = constp.tile([1, C, D], F32)
            nc.sync.dma_start(cw[:], cw_d[:, :])
            cb = constp.tile([1, C], F32)
            nc.sync.dma_start(cb[:], cb_d[:, :])
            idx = constp.tile([128, BPC, SBLK], I32)
            for b in range(BPC):
                nc.sync.dma_start(idx[:, b, :], tok_d[b, :, :])

            for b in range(BPC):
                # Gather the 2048 embedding rows for this batch element.
                # Token t lands on partition t%128, free block t//128; one
                # indirect DMA per 128-token block (one index per partition).
                x = xp.tile([128, SBLK, D], BF16, tag="x")
                for j in range(SBLK):
                    nc.gpsimd.indirect_dma_start(
                        out=x[:, j, :],
                        out_offset=None,
                        in_=emb_d[:, :],
                        in_offset=bass.IndirectOffsetOnAxis(
                            ap=idx[:, b, j : j + 1], axis=0
                        ),
                    )

                # Broadcast q = x[token 0] to all 128 partitions via a K=1
                # outer-product matmul: ones[1,128]^T @ x[0:1, 0, :].
                qb = pp.tile([128, D], F32, tag="qb")
                nc.tensor.matmul(qb[:], ones1[:], x[0:1, 0, :], start=True, stop=True)
                qbs = sp.tile([128, D], BF16, tag="qbs")
                nc.scalar.copy(qbs[:], qb[:])

                # Scores s[t] = <x_t, q>: fused multiply+reduce per block
                # ((x*1) * q with accum_out = row sums).
                s = sp.tile([128, SBLK], F32, tag="s")
                for j in range(SBLK):
                    junk = jp.tile([128, D], BF16, tag="junk")
                    nc.vector.scalar_tensor_tensor(
                        out=junk[:],
                        in0=x[:, j, :],
                        scalar=1.0,
                        in1=qbs[:],
                        op0=mult,
                        op1=mult,
                        accum_out=s[:, j : j + 1],
                    )

                # a = exp(s) (scores are O(0.2): no max subtraction needed),
                # with fused per-partition row sums for the softmax denom.
                e = sp.tile([128, SBLK], BF16, tag="e")
                zcol = sp.tile([128, 1], F32, tag="zcol")
                nc.scalar.activation(e[:], s[:], EXP, accum_out=zcol[:])

                # pooled = a^T X (unnormalized), accumulated over blocks.
                pooled = pp.tile([1, D], F32, tag="pooled")
                for j in range(SBLK):
                    nc.tensor.matmul(
                        pooled[:],
                        e[:, j : j + 1],
                        x[:, j, :],
                        start=(j == 0),
                        stop=(j == SBLK - 1),
                    )

                # Z = sum over partitions of zcol.
                zps = pp.tile([1, 1], F32, tag="z")
                nc.tensor.matmul(zps[:], zcol[:], ones128[:, :], start=True, stop=True)

                psb = tp.tile([1, D], F32, tag="psb")
                nc.scalar.copy(psb[:], pooled[:])
                zsb = tp.tile([1, 1], F32, tag="zsb")
                nc.vector.tensor_copy(zsb[:], zps[:])
                rz = tp.tile([1, 1], F32, tag="rz")
                nc.vector.reciprocal(rz[:], zsb[:])

                # logits_c = <pooled, cls_w_c>
                lg = tp.tile([1, C], F32, tag="lg")
                for c in range(C):
                    junk2 = tp.tile([1, D], F32, tag="junk2")
                    nc.vector.scalar_tensor_tensor(
                        out=junk2[:],
                        in0=psb[:],
                        scalar=1.0,
                        in1=cw[:, c, :],
                        op0=mult,
                        op1=mult,
                        accum_out=lg[:, c : c + 1],
                    )

                # out = lg / Z + cls_b
                ob = tp.tile([1, C], F32, tag="ob")
                nc.vector.scalar_tensor_tensor(
                    ob[:], lg[:], rz[:], cb[:], op0=mult, op1=add
                )
                nc.sync.dma_start(out_d[b : b + 1, :], ob[:])

    nc.finalize()
    if split:
        _split_multiwaits(nc)
    return nc


def _wrap_tokens(tokens_row: np.ndarray) -> np.ndarray:
    """[S] int tokens -> [128, SBLK] int32; token t at [t%128, t//128]."""
    return np.ascontiguousarray(tokens_row.reshape(SBLK, 128).T.astype(np.int32))


def get_nc() -> bass.Bass:
    if "nc" not in _CACHE:
        _CACHE["nc"] = _build_nc()
    return _CACHE["nc"]


def make_in_maps(tokens, emb_table, cls_w, cls_b):
    import ml_dtypes

    tokens = np.asarray(tokens)
    emb = np.ascontiguousarray(
        np.asarray(emb_table, dtype=np.float32).astype(ml_dtypes.bfloat16)
    )
    cw = np.ascontiguousarray(np.asarray(cls_w, dtype=np.float32)).reshape(1, C * D)
    cb = np.ascontiguousarray(np.asarray(cls_b, dtype=np.float32)).reshape(1, C)
    idx_all = np.stack([_wrap_tokens(tokens[b]) for b in range(B)])  # [B,128,IDXW]
    in_maps = []
    for core in range(N_CORES):
        in_maps.append(
            {
                "emb": emb,
                "tok": idx_all[core * BPC : (core + 1) * BPC],
                "cls_w": cw,
                "cls_b": cb,
            }
        )
    return in_maps


def kernel(tokens, emb_table, cls_w, cls_b) -> np.ndarray:
    nc = get_nc()
    in_maps = make_in_maps(tokens, emb_table, cls_w, cls_b)
    res = run_bass_kernel_spmd(nc, in_maps, core_ids=list(range(N_CORES)))
    outs = [res.results[c]["out"] for c in range(N_CORES)]
    return np.concatenate(outs, axis=0).astype(np.float32)



# revision 30
# speedup vs baseline: 1.2189x; 1.0894x over previous
"""Trainium2 Bass kernel for nn_AttentionModel (sparse_attention).

Reference computation:
    x = emb_table[tokens]                  # [B,S,D]
    scores = x @ x^T per batch             # [B,S,S]
    out = softmax(scores) @ x              # [B,S,D]
    logits = out[:, 0, :] @ cls_w.T + cls_b

Only row 0 of the attention output is used, so per batch element we only
need attention row 0:
    q = x[0]; s = X q; a = exp(s); logits = (a^T X / sum(a)) cls_w^T + cls_b
This turns ~275 GFLOP of full attention into an embedding gather plus
~4 MFLOP per batch element.

Sharding: data-parallel over batch. 8 cores x 4 batch elements; emb_table
and classifier weights replicated; no collectives.
"""

import numpy as np

import bass_rust

import concourse.bass as bass
import concourse.mybir as mybir
import concourse.tile as tile
from concourse.bass_utils import run_bass_kernel_spmd


def _split_multiwaits(nc: bass.Bass) -> None:
    """Workaround for the walrus build in this container, which rejects
    instructions carrying more than one sync-wait command ("Too many sync
    wait commands" / "ISA wrong length" in CoreV3GenImpl setupSyncWait).

    Moves each instruction's sync waits onto dedicated single-wait NOPs
    inserted right before it on the same engine stream (bass_nofuse so
    walrus's nop-fusion can't merge them back). Semantics are identical:
    the engine's sequencer blocks on the NOPs' waits, then issues the
    now-waitless instruction. Sem updates stay on the original."""
    counter = 0
    fn = nc.m.functions[0]
    for bb in fn.blocks:
        insts = bb.instructions
        new_list = []
        changed = False
        for inst in insts:
            si = inst.sync_info
            waits = list(si.on_wait) if si is not None else []
            if waits:
                for w in waits:
                    counter += 1
                    new_list.append(
                        mybir.InstNoOp(
                            name=f"waitnop-{counter}",
                            engine=inst.engine,
                            ins=[],
                            outs=[],
                            bass_nofuse=True,
                            sync_info=bass_rust.SyncInfo(on_wait=[w], on_update=[]),
                        )
                    )
                inst.sync_info = bass_rust.SyncInfo(
                    on_wait=[], on_update=list(si.on_update)
                )
                changed = True
            new_list.append(inst)
        if changed:
            bb.instructions = new_list

B, S, D, V, C = 32, 2048, 512, 32000, 2
N_CORES = 8
BPC = B // N_CORES          # batch elements per core
SBLK = S // 128             # 16 free-dim blocks of gathered tokens

F32 = mybir.dt.float32
BF16 = mybir.dt.bfloat16
I32 = mybir.dt.int32

_CACHE: dict = {}


def _build_nc(split: bool = True) -> bass.Bass:
    nc = bass.Bass()
    emb_d = nc.dram_tensor("emb", [V, D], BF16, kind="ExternalInput")
    tok_d = nc.dram_tensor("tok", [128, BPC * SBLK], I32, kind="ExternalInput")
    cw_d = nc.dram_tensor("cls_w", [1, C * D], F32, kind="ExternalInput")
    cb_d = nc.dram_tensor("cls_b", [1, C], F32, kind="ExternalInput")
    out_d = nc.dram_tensor("out", [BPC, C], F32, kind="ExternalOutput")

    mult = mybir.AluOpType.mult
    add = mybir.AluOpType.add
    EXP = mybir.ActivationFunctionType.Exp

    with tile.TileContext(nc) as tc:
        with (
            tc.tile_pool(name="const", bufs=1) as constp,
            tc.tile_pool(name="xp", bufs=BPC) as xp,
            tc.tile_pool(name="sp", bufs=2) as sp,
            tc.tile_pool(name="jp", bufs=2) as jp,
            tc.tile_pool(name="tp", bufs=2) as tp,
            tc.tile_pool(name="ps", bufs=2, space="PSUM") as pp,
        ):
            idx = constp.tile([128, BPC, SBLK], I32)
            nc.sync.dma_start(idx[:, :, :], tok_d[:, :])
            ones1 = constp.tile([1, 128], BF16)
            nc.vector.memset(ones1[:], 1.0)
            ones128 = constp.tile([128, 1], F32)
            nc.vector.memset(ones128[:], 1.0)
            cw = constp.tile([1, C, D], F32)
            nc.sync.dma_start(cw[:], cw_d[:, :])
            cb = constp.tile([1, C], F32)
            nc.sync.dma_start(cb[:], cb_d[:, :])
            for b in range(BPC):
                # Gather the 2048 embedding rows for this batch element.
                # Token t lands on partition t%128, free block t//128; one
                # indirect DMA per 128-token block (one index per partition).
                x = xp.tile([128, SBLK, D], BF16, tag="x")
                for j in range(SBLK):
                    nc.gpsimd.indirect_dma_start(
                        out=x[:, j, :],
                        out_offset=None,
                        in_=emb_d[:, :],
                        in_offset=bass.IndirectOffsetOnAxis(
                            ap=idx[:, b, j : j + 1], axis=0
                        ),
                    )

                # Broadcast q = x[token 0] to all 128 partitions via a K=1
                # outer-product matmul: ones[1,128]^T @ x[0:1, 0, :].
                qb = pp.tile([128, D], F32, tag="qb")
                nc.tensor.matmul(qb[:], ones1[:], x[0:1, 0, :], start=True, stop=True)
                qbs = sp.tile([128, D], BF16, tag="qbs")
                nc.scalar.copy(qbs[:], qb[:])

                # Scores s[t] = <x_t, q>: fused multiply+reduce per block
                # ((x*1) * q with accum_out = row sums).
                s = sp.tile([128, SBLK], F32, tag="s")
                for j in range(SBLK):
                    junk = jp.tile([128, D], BF16, tag="junk")
                    nc.vector.scalar_tensor_tensor(
                        out=junk[:],
                        in0=x[:, j, :],
                        scalar=1.0,
                        in1=qbs[:],
                        op0=mult,
                        op1=mult,
                        accum_out=s[:, j : j + 1],
                    )

                # a = exp(s) (scores are O(0.2): no max subtraction needed),
                # with fused per-partition row sums for the softmax denom.
                # Split into groups of 4 blocks so the pooled chain becomes
                # ready incrementally (tracks the gather stream, keeps PE
                # warm, and shrinks the end-of-kernel tail to one group).
                EG = 4
                e = sp.tile([128, SBLK], BF16, tag="e")
                zcols = sp.tile([128, EG], F32, tag="zcols")
                for g in range(EG):
                    lo, hi = g * (SBLK // EG), (g + 1) * (SBLK // EG)
                    nc.scalar.activation(
                        e[:, lo:hi], s[:, lo:hi], EXP,
                        accum_out=zcols[:, g : g + 1],
                    )

                # pooled = a^T X (unnormalized), accumulated over blocks.
                pooled = pp.tile([1, D], F32, tag="pooled")
                for j in range(SBLK):
                    nc.tensor.matmul(
                        pooled[:],
                        e[:, j : j + 1],
                        x[:, j, :],
                        start=(j == 0),
                        stop=(j == SBLK - 1),
                    )

                # Z = sum over partitions of all zcol groups.
                zps = pp.tile([1, 1], F32, tag="z")
                for g in range(EG):
                    nc.tensor.matmul(
                        zps[:], zcols[:, g : g + 1], ones128[:, :],
                        start=(g == 0), stop=(g == EG - 1),
                    )

                psb = tp.tile([1, D], F32, tag="psb")
                nc.scalar.copy(psb[:], pooled[:])
                zsb = tp.tile([1, 1], F32, tag="zsb")
                nc.vector.tensor_copy(zsb[:], zps[:])
                rz = tp.tile([1, 1], F32, tag="rz")
                nc.vector.reciprocal(rz[:], zsb[:])

                # logits_c = <pooled, cls_w_c>
                lg = tp.tile([1, C], F32, tag="lg")
                for c in range(C):
                    junk2 = tp.tile([1, D], F32, tag="junk2")
                    nc.vector.scalar_tensor_tensor(
                        out=junk2[:],
                        in0=psb[:],
                        scalar=1.0,
                        in1=cw[:, c, :],
                        op0=mult,
                        op1=mult,
                        accum_out=lg[:, c : c + 1],
                    )

                # out = lg / Z + cls_b
                ob = tp.tile([1, C], F32, tag="ob")
                nc.vector.scalar_tensor_tensor(
                    ob[:], lg[:], rz[:], cb[:], op0=mult, op1=add
                )
                nc.sync.dma_start(out_d[b : b + 1, :], ob[:])

    nc.finalize()
    if split:
        _split_multiwaits(nc)
    return nc


def _wrap_tokens(tokens_row: np.ndarray) -> np.ndarray:
    """[S] int tokens -> [128, SBLK] int32; token t at [t%128, t//128]."""
    return np.ascontiguousarray(tokens_row.reshape(SBLK, 128).T.astype(np.int32))


def get_nc() -> bass.Bass:
    if "nc" not in _CACHE:
        _CACHE["nc"] = _build_nc()
    return _CACHE["nc"]


def make_in_maps(tokens, emb_table, cls_w, cls_b):
    import ml_dtypes

    tokens = np.asarray(tokens)
    emb = np.ascontiguousarray(
        np.asarray(emb_table, dtype=np.float32).astype(ml_dtypes.bfloat16)
    )
    cw = np.ascontiguousarray(np.asarray(cls_w, dtype=np.float32)).reshape(1, C * D)
    cb = np.ascontiguousarray(np.asarray(cls_b, dtype=np.float32)).reshape(1, C)
    in_maps = []
    for core in range(N_CORES):
        idx_flat = np.concatenate(
            [_wrap_tokens(tokens[core * BPC + b]) for b in range(BPC)], axis=1
        )  # [128, BPC*SBLK]
        in_maps.append(
            {
                "emb": emb,
                "tok": np.ascontiguousarray(idx_flat),
                "cls_w": cw,
                "cls_b": cb,
            }
        )
    return in_maps


def kernel(tokens, emb_table, cls_w, cls_b) -> np.ndarray:
    nc = get_nc()
    in_maps = make_in_maps(tokens, emb_table, cls_w, cls_b)
    res = run_bass_kernel_spmd(nc, in_maps, core_ids=list(range(N_CORES)))
    outs = [res.results[c]["out"] for c in range(N_CORES)]
    return np.concatenate(outs, axis=0).astype(np.float32)



# revision 31
# speedup vs baseline: 1.2356x; 1.0137x over previous
"""Trainium2 Bass kernel for nn_AttentionModel (sparse_attention).

Reference computation:
    x = emb_table[tokens]                  # [B,S,D]
    scores = x @ x^T per batch             # [B,S,S]
    out = softmax(scores) @ x              # [B,S,D]
    logits = out[:, 0, :] @ cls_w.T + cls_b

Only row 0 of the attention output is used, so per batch element we only
need attention row 0:
    q = x[0]; s = X q; a = exp(s); logits = (a^T X / sum(a)) cls_w^T + cls_b
This turns ~275 GFLOP of full attention into an embedding gather plus
~4 MFLOP per batch element.

Sharding: data-parallel over batch. 8 cores x 4 batch elements; emb_table
and classifier weights replicated; no collectives.
"""

import numpy as np

import bass_rust

import concourse.bass as bass
import concourse.mybir as mybir
import concourse.tile as tile
from concourse.bass_utils import run_bass_kernel_spmd


def _split_multiwaits(nc: bass.Bass) -> None:
    """Workaround for the walrus build in this container, which rejects
    instructions carrying more than one sync-wait command ("Too many sync
    wait commands" / "ISA wrong length" in CoreV3GenImpl setupSyncWait).

    Moves each instruction's sync waits onto dedicated single-wait NOPs
    inserted right before it on the same engine stream (bass_nofuse so
    walrus's nop-fusion can't merge them back). Semantics are identical:
    the engine's sequencer blocks on the NOPs' waits, then issues the
    now-waitless instruction. Sem updates stay on the original."""
    counter = 0
    fn = nc.m.functions[0]
    for bb in fn.blocks:
        insts = bb.instructions
        new_list = []
        changed = False
        for inst in insts:
            si = inst.sync_info
            waits = list(si.on_wait) if si is not None else []
            if waits:
                for w in waits:
                    counter += 1
                    new_list.append(
                        mybir.InstNoOp(
                            name=f"waitnop-{counter}",
                            engine=inst.engine,
                            ins=[],
                            outs=[],
                            bass_nofuse=True,
                            sync_info=bass_rust.SyncInfo(on_wait=[w], on_update=[]),
                        )
                    )
                inst.sync_info = bass_rust.SyncInfo(
                    on_wait=[], on_update=list(si.on_update)
                )
                changed = True
            new_list.append(inst)
        if changed:
            bb.instructions = new_list

B, S, D, V, C = 32, 2048, 512, 32000, 2
N_CORES = 8
BPC = B // N_CORES          # batch elements per core
SBLK = S // 128             # 16 free-dim blocks of gathered tokens

F32 = mybir.dt.float32
BF16 = mybir.dt.bfloat16
I32 = mybir.dt.int32

_CACHE: dict = {}


def _build_nc(split: bool = True) -> bass.Bass:
    nc = bass.Bass()
    emb_d = nc.dram_tensor("emb", [V, D], BF16, kind="ExternalInput")
    tok_d = nc.dram_tensor("tok", [128, BPC * SBLK], I32, kind="ExternalInput")
    cw_d = nc.dram_tensor("cls_w", [C, D], BF16, kind="ExternalInput")
    cb_d = nc.dram_tensor("cls_b", [C, 1], F32, kind="ExternalInput")
    out_d = nc.dram_tensor("out", [BPC, C], F32, kind="ExternalOutput")

    mult = mybir.AluOpType.mult
    add = mybir.AluOpType.add
    EXP = mybir.ActivationFunctionType.Exp

    with tile.TileContext(nc) as tc:
        with (
            tc.tile_pool(name="const", bufs=1) as constp,
            tc.tile_pool(name="xp", bufs=BPC) as xp,
            tc.tile_pool(name="sp", bufs=2) as sp,
            tc.tile_pool(name="jp", bufs=2) as jp,
            tc.tile_pool(name="tp", bufs=2) as tp,
            tc.tile_pool(name="ps", bufs=2, space="PSUM") as pp,
        ):
            idx = constp.tile([128, BPC, SBLK], I32)
            nc.sync.dma_start(idx[:, :, :], tok_d[:, :])
            ones1 = constp.tile([1, 128], BF16)
            nc.vector.memset(ones1[:], 1.0)
            ones128 = constp.tile([128, 1], F32)
            nc.vector.memset(ones128[:], 1.0)
            cw = constp.tile([C, D], BF16)
            nc.sync.dma_start(cw[:], cw_d[:, :])
            cb = constp.tile([C, 1], F32)
            nc.sync.dma_start(cb[:], cb_d[:, :])
            for b in range(BPC):
                # Gather the 2048 embedding rows for this batch element.
                # Token t lands on partition t%128, free block t//128; one
                # indirect DMA per 128-token block (one index per partition).
                x = xp.tile([128, SBLK, D], BF16, tag="x")
                for j in range(SBLK):
                    nc.gpsimd.indirect_dma_start(
                        out=x[:, j, :],
                        out_offset=None,
                        in_=emb_d[:, :],
                        in_offset=bass.IndirectOffsetOnAxis(
                            ap=idx[:, b, j : j + 1], axis=0
                        ),
                    )

                # Broadcast q = x[token 0] to all 128 partitions via a K=1
                # outer-product matmul: ones[1,128]^T @ x[0:1, 0, :].
                qb = pp.tile([128, D], F32, tag="qb")
                nc.tensor.matmul(qb[:], ones1[:], x[0:1, 0, :], start=True, stop=True)
                qbs = sp.tile([128, D], BF16, tag="qbs")
                nc.scalar.copy(qbs[:], qb[:])

                # Scores s[t] = <x_t, q>: fused multiply+reduce per block
                # ((x*1) * q with accum_out = row sums).
                s = sp.tile([128, SBLK], F32, tag="s")
                for j in range(SBLK):
                    junk = jp.tile([128, D], BF16, tag="junk")
                    nc.vector.scalar_tensor_tensor(
                        out=junk[:],
                        in0=x[:, j, :],
                        scalar=1.0,
                        in1=qbs[:],
                        op0=mult,
                        op1=mult,
                        accum_out=s[:, j : j + 1],
                    )

                # a = exp(s) (scores are O(0.2): no max subtraction needed),
                # with fused per-partition row sums for the softmax denom.
                # Split into groups of 4 blocks so the pooled chain becomes
                # ready incrementally (tracks the gather stream, keeps PE
                # warm, and shrinks the end-of-kernel tail to one group).
                EG = 4
                e = sp.tile([128, SBLK], BF16, tag="e")
                zcols = sp.tile([128, EG], F32, tag="zcols")
                for g in range(EG):
                    lo, hi = g * (SBLK // EG), (g + 1) * (SBLK // EG)
                    nc.scalar.activation(
                        e[:, lo:hi], s[:, lo:hi], EXP,
                        accum_out=zcols[:, g : g + 1],
                    )

                # pooled = a^T X (unnormalized), accumulated over blocks,
                # replicated to C partitions via lhsT free-dim broadcast.
                pooled = pp.tile([C, D], F32, tag="pooled")
                for j in range(SBLK):
                    nc.tensor.matmul(
                        pooled[:],
                        e[:, j : j + 1].broadcast_to([128, C]),
                        x[:, j, :],
                        start=(j == 0),
                        stop=(j == SBLK - 1),
                    )

                # Z = sum over partitions of all zcol groups, C-replicated.
                zps = pp.tile([C, 1], F32, tag="z")
                for g in range(EG):
                    nc.tensor.matmul(
                        zps[:], zcols[:, g : g + 1].broadcast_to([128, C]),
                        ones128[:, :],
                        start=(g == 0), stop=(g == EG - 1),
                    )

                psb = tp.tile([C, D], BF16, tag="psb")
                nc.scalar.copy(psb[:], pooled[:])
                zsb = tp.tile([C, 1], F32, tag="zsb")
                nc.scalar.copy(zsb[:], zps[:])
                rz = tp.tile([C, 1], F32, tag="rz")
                nc.vector.reciprocal(rz[:], zsb[:])

                # logits_c = <pooled, cls_w_c>: one fused dot over C rows
                junk2 = tp.tile([C, D], BF16, tag="junk2")
                lg = tp.tile([C, 1], F32, tag="lg")
                nc.vector.scalar_tensor_tensor(
                    out=junk2[:],
                    in0=psb[:],
                    scalar=1.0,
                    in1=cw[:],
                    op0=mult,
                    op1=mult,
                    accum_out=lg[:],
                )

                # out = lg / Z + cls_b
                ob = tp.tile([C, 1], F32, tag="ob")
                nc.vector.scalar_tensor_tensor(
                    ob[:], lg[:], rz[:], cb[:], op0=mult, op1=add
                )
                nc.sync.dma_start(out_d[b, :].unsqueeze(1), ob[:, :])

    nc.finalize()
    if split:
        _split_multiwaits(nc)
    return nc


def _wrap_tokens(tokens_row: np.ndarray) -> np.ndarray:
    """[S] int tokens -> [128, SBLK] int32; token t at [t%128, t//128]."""
    return np.ascontiguousarray(tokens_row.reshape(SBLK, 128).T.astype(np.int32))


def get_nc() -> bass.Bass:
    if "nc" not in _CACHE:
        _CACHE["nc"] = _build_nc()
    return _CACHE["nc"]


def make_in_maps(tokens, emb_table, cls_w, cls_b):
    import ml_dtypes

    tokens = np.asarray(tokens)
    emb = np.ascontiguousarray(
        np.asarray(emb_table, dtype=np.float32).astype(ml_dtypes.bfloat16)
    )
    cw = np.ascontiguousarray(
        np.asarray(cls_w, dtype=np.float32).astype(ml_dtypes.bfloat16)
    )
    cb = np.ascontiguousarray(np.asarray(cls_b, dtype=np.float32).reshape(C, 1))
    in_maps = []
    for core in range(N_CORES):
        idx_flat = np.concatenate(
            [_wrap_tokens(tokens[core * BPC + b]) for b in range(BPC)], axis=1
        )  # [128, BPC*SBLK]
        in_maps.append(
            {
                "emb": emb,
                "tok": np.ascontiguousarray(idx_flat),
                "cls_w": cw,
                "cls_b": cb,
            }
        )
    return in_maps


def kernel(tokens, emb_table, cls_w, cls_b) -> np.ndarray:
    nc = get_nc()
    in_maps = make_in_maps(tokens, emb_table, cls_w, cls_b)
    res = run_bass_kernel_spmd(nc, in_maps, core_ids=list(range(N_CORES)))
    outs = [res.results[c]["out"] for c in range(N_CORES)]
    return np.concatenate(outs, axis=0).astype(np.float32)



# revision 33
# speedup vs baseline: 1.2496x; 1.0113x over previous
"""Trainium2 Bass kernel for nn_AttentionModel (sparse_attention).

Reference computation:
    x = emb_table[tokens]                  # [B,S,D]
    scores = x @ x^T per batch             # [B,S,S]
    out = softmax(scores) @ x              # [B,S,D]
    logits = out[:, 0, :] @ cls_w.T + cls_b

Only row 0 of the attention output is used, so per batch element we only
need attention row 0:
    q = x[0]; s = X q; a = exp(s); logits = (a^T X / sum(a)) cls_w^T + cls_b
This turns ~275 GFLOP of full attention into an embedding gather plus
~4 MFLOP per batch element.

Sharding: data-parallel over batch. 8 cores x 4 batch elements; emb_table
and classifier weights replicated; no collectives.
"""

import numpy as np

import bass_rust

import concourse.bass as bass
import concourse.mybir as mybir
import concourse.tile as tile
from concourse.bass_utils import run_bass_kernel_spmd


def _split_multiwaits(nc: bass.Bass) -> None:
    """Workaround for the walrus build in this container, which rejects
    instructions carrying more than one sync-wait command ("Too many sync
    wait commands" / "ISA wrong length" in CoreV3GenImpl setupSyncWait).

    Moves each instruction's sync waits onto dedicated single-wait NOPs
    inserted right before it on the same engine stream (bass_nofuse so
    walrus's nop-fusion can't merge them back). Semantics are identical:
    the engine's sequencer blocks on the NOPs' waits, then issues the
    now-waitless instruction. Sem updates stay on the original."""
    counter = 0
    fn = nc.m.functions[0]
    for bb in fn.blocks:
        insts = bb.instructions
        new_list = []
        changed = False
        for inst in insts:
            si = inst.sync_info
            waits = list(si.on_wait) if si is not None else []
            if waits:
                for w in waits:
                    counter += 1
                    new_list.append(
                        mybir.InstNoOp(
                            name=f"waitnop-{counter}",
                            engine=inst.engine,
                            ins=[],
                            outs=[],
                            bass_nofuse=True,
                            sync_info=bass_rust.SyncInfo(on_wait=[w], on_update=[]),
                        )
                    )
                inst.sync_info = bass_rust.SyncInfo(
                    on_wait=[], on_update=list(si.on_update)
                )
                changed = True
            new_list.append(inst)
        if changed:
            bb.instructions = new_list

B, S, D, V, C = 32, 2048, 512, 32000, 2
N_CORES = 8
BPC = B // N_CORES          # batch elements per core
SBLK = S // 128             # 16 free-dim blocks of gathered tokens

F32 = mybir.dt.float32
BF16 = mybir.dt.bfloat16
I32 = mybir.dt.int32

_CACHE: dict = {}


def _build_nc(split: bool = True) -> bass.Bass:
    nc = bass.Bass()
    emb_d = nc.dram_tensor("emb", [V, D], BF16, kind="ExternalInput")
    tok_d = nc.dram_tensor("tok", [128, BPC * SBLK], I32, kind="ExternalInput")
    cw_d = nc.dram_tensor("cls_w", [C, D], BF16, kind="ExternalInput")
    cb_d = nc.dram_tensor("cls_b", [C, 1], F32, kind="ExternalInput")
    out_d = nc.dram_tensor("out", [BPC, C], F32, kind="ExternalOutput")

    mult = mybir.AluOpType.mult
    add = mybir.AluOpType.add
    EXP = mybir.ActivationFunctionType.Exp

    with tile.TileContext(nc) as tc:
        with (
            tc.tile_pool(name="const", bufs=1) as constp,
            tc.tile_pool(name="xp", bufs=BPC) as xp,
            tc.tile_pool(name="sp", bufs=2) as sp,
            tc.tile_pool(name="jp", bufs=2) as jp,
            tc.tile_pool(name="tp", bufs=2) as tp,
            tc.tile_pool(name="ps", bufs=2, space="PSUM") as pp,
        ):
            idx = constp.tile([128, BPC, SBLK], I32)
            nc.sync.dma_start(idx[:, :, :], tok_d[:, :])
            ones1 = constp.tile([1, 128], BF16)
            nc.vector.memset(ones1[:], 1.0)
            ones128 = constp.tile([128, 1], F32)
            nc.vector.memset(ones128[:], 1.0)
            cw = constp.tile([C, D], BF16)
            nc.sync.dma_start(cw[:], cw_d[:, :])
            cb = constp.tile([C, 1], F32)
            nc.sync.dma_start(cb[:], cb_d[:, :])
            for b in range(BPC):
                # Gather the 2048 embedding rows for this batch element.
                # Token t lands on partition t%128, free block t//128; one
                # indirect DMA per 128-token block (one index per partition).
                x = xp.tile([128, SBLK, D], BF16, tag="x")
                for j in range(SBLK):
                    nc.gpsimd.indirect_dma_start(
                        out=x[:, j, :],
                        out_offset=None,
                        in_=emb_d[:, :],
                        in_offset=bass.IndirectOffsetOnAxis(
                            ap=idx[:, b, j : j + 1], axis=0
                        ),
                    )

                # Broadcast q = x[token 0] to all 128 partitions via a K=1
                # outer-product matmul: ones[1,128]^T @ x[0:1, 0, :].
                qb = pp.tile([128, D], F32, tag="qb")
                nc.tensor.matmul(qb[:], ones1[:], x[0:1, 0, :], start=True, stop=True)
                qbs = sp.tile([128, D], BF16, tag="qbs")
                nc.scalar.copy(qbs[:], qb[:])

                # Scores s[t] = <x_t, q>: fused multiply+reduce per block
                # ((x*1) * q with accum_out = row sums).
                s = sp.tile([128, SBLK], F32, tag="s")
                for j in range(SBLK):
                    junk = jp.tile([128, D], BF16, tag="junk")
                    nc.vector.scalar_tensor_tensor(
                        out=junk[:],
                        in0=x[:, j, :],
                        scalar=1.0,
                        in1=qbs[:],
                        op0=mult,
                        op1=mult,
                        accum_out=s[:, j : j + 1],
                    )

                # a = exp(s) (scores are O(0.2): no max subtraction needed),
                # with fused per-partition row sums for the softmax denom.
                # Split into groups of 4 blocks so the pooled chain becomes
                # ready incrementally (tracks the gather stream, keeps PE
                # warm, and shrinks the end-of-kernel tail to one group).
                EG = 16
                e = sp.tile([128, SBLK], BF16, tag="e")
                zcols = sp.tile([128, EG], F32, tag="zcols")
                for g in range(EG):
                    lo, hi = g * (SBLK // EG), (g + 1) * (SBLK // EG)
                    nc.scalar.activation(
                        e[:, lo:hi], s[:, lo:hi], EXP,
                        accum_out=zcols[:, g : g + 1],
                    )

                # pooled = a^T X (unnormalized), accumulated over blocks,
                # replicated to C partitions via lhsT free-dim broadcast.
                pooled = pp.tile([C, D], F32, tag="pooled")
                for j in range(SBLK):
                    nc.tensor.matmul(
                        pooled[:],
                        e[:, j : j + 1].broadcast_to([128, C]),
                        x[:, j, :],
                        start=(j == 0),
                        stop=(j == SBLK - 1),
                    )

                # Z = sum over partitions of all zcol groups, C-replicated.
                zps = pp.tile([C, 1], F32, tag="z")
                for g in range(EG):
                    nc.tensor.matmul(
                        zps[:], zcols[:, g : g + 1].broadcast_to([128, C]),
                        ones128[:, :],
                        start=(g == 0), stop=(g == EG - 1),
                    )

                psb = tp.tile([C, D], BF16, tag="psb")
                nc.scalar.copy(psb[:], pooled[:])
                zsb = tp.tile([C, 1], F32, tag="zsb")
                nc.scalar.copy(zsb[:], zps[:])
                rz = tp.tile([C, 1], F32, tag="rz")
                nc.vector.reciprocal(rz[:], zsb[:])

                # logits_c = <pooled, cls_w_c>: one fused dot over C rows
                junk2 = tp.tile([C, D], BF16, tag="junk2")
                lg = tp.tile([C, 1], F32, tag="lg")
                nc.vector.scalar_tensor_tensor(
                    out=junk2[:],
                    in0=psb[:],
                    scalar=1.0,
                    in1=cw[:],
                    op0=mult,
                    op1=mult,
                    accum_out=lg[:],
                )

                # out = lg / Z + cls_b
                ob = tp.tile([C, 1], F32, tag="ob")
                nc.vector.scalar_tensor_tensor(
                    ob[:], lg[:], rz[:], cb[:], op0=mult, op1=add
                )
                nc.sync.dma_start(out_d[b, :].unsqueeze(1), ob[:, :])

    nc.finalize()
    if split:
        _split_multiwaits(nc)
    return nc


def _wrap_tokens(tokens_row: np.ndarray) -> np.ndarray:
    """[S] int tokens -> [128, SBLK] int32; token t at [t%128, t//128]."""
    return np.ascontiguousarray(tokens_row.reshape(SBLK, 128).T.astype(np.int32))


def get_nc() -> bass.Bass:
    if "nc" not in _CACHE:
        _CACHE["nc"] = _build_nc()
    return _CACHE["nc"]


def make_in_maps(tokens, emb_table, cls_w, cls_b):
    import ml_dtypes

    tokens = np.asarray(tokens)
    emb = np.ascontiguousarray(
        np.asarray(emb_table, dtype=np.float32).astype(ml_dtypes.bfloat16)
    )
    cw = np.ascontiguousarray(
        np.asarray(cls_w, dtype=np.float32).astype(ml_dtypes.bfloat16)
    )
    cb = np.ascontiguousarray(np.asarray(cls_b, dtype=np.float32).reshape(C, 1))
    in_maps = []
    for core in range(N_CORES):
        idx_flat = np.concatenate(
            [_wrap_tokens(tokens[core * BPC + b]) for b in range(BPC)], axis=1
        )  # [128, BPC*SBLK]
        in_maps.append(
            {
                "emb": emb,
                "tok": np.ascontiguousarray(idx_flat),
                "cls_w": cw,
                "cls_b": cb,
            }
        )
    return in_maps


def kernel(tokens, emb_table, cls_w, cls_b) -> np.ndarray:
    nc = get_nc()
    in_maps = make_in_maps(tokens, emb_table, cls_w, cls_b)
    res = run_bass_kernel_spmd(nc, in_maps, core_ids=list(range(N_CORES)))
    outs = [res.results[c]["out"] for c in range(N_CORES)]
    return np.concatenate(outs, axis=0).astype(np.float32)



# revision 34
# speedup vs baseline: 1.2633x; 1.0109x over previous
"""Trainium2 Bass kernel for nn_AttentionModel (sparse_attention).

Reference computation:
    x = emb_table[tokens]                  # [B,S,D]
    scores = x @ x^T per batch             # [B,S,S]
    out = softmax(scores) @ x              # [B,S,D]
    logits = out[:, 0, :] @ cls_w.T + cls_b

Only row 0 of the attention output is used, so per batch element we only
need attention row 0:
    q = x[0]; s = X q; a = exp(s); logits = (a^T X / sum(a)) cls_w^T + cls_b
This turns ~275 GFLOP of full attention into an embedding gather plus
~4 MFLOP per batch element.

Sharding: data-parallel over batch. 8 cores x 4 batch elements; emb_table
and classifier weights replicated; no collectives.
"""

import numpy as np

import bass_rust

import concourse.bass as bass
import concourse.mybir as mybir
import concourse.tile as tile
from concourse.bass_utils import run_bass_kernel_spmd


def _split_multiwaits(nc: bass.Bass) -> None:
    """Workaround for the walrus build in this container, which rejects
    instructions carrying more than one sync-wait command ("Too many sync
    wait commands" / "ISA wrong length" in CoreV3GenImpl setupSyncWait).

    Moves each instruction's sync waits onto dedicated single-wait NOPs
    inserted right before it on the same engine stream (bass_nofuse so
    walrus's nop-fusion can't merge them back). Semantics are identical:
    the engine's sequencer blocks on the NOPs' waits, then issues the
    now-waitless instruction. Sem updates stay on the original."""
    counter = 0
    fn = nc.m.functions[0]
    for bb in fn.blocks:
        insts = bb.instructions
        new_list = []
        changed = False
        for inst in insts:
            si = inst.sync_info
            waits = list(si.on_wait) if si is not None else []
            if waits:
                for w in waits:
                    counter += 1
                    new_list.append(
                        mybir.InstNoOp(
                            name=f"waitnop-{counter}",
                            engine=inst.engine,
                            ins=[],
                            outs=[],
                            bass_nofuse=True,
                            sync_info=bass_rust.SyncInfo(on_wait=[w], on_update=[]),
                        )
                    )
                inst.sync_info = bass_rust.SyncInfo(
                    on_wait=[], on_update=list(si.on_update)
                )
                changed = True
            new_list.append(inst)
        if changed:
            bb.instructions = new_list

B, S, D, V, C = 32, 2048, 512, 32000, 2
N_CORES = 8
BPC = B // N_CORES          # batch elements per core
SBLK = S // 128             # 16 free-dim blocks of gathered tokens

F32 = mybir.dt.float32
BF16 = mybir.dt.bfloat16
I32 = mybir.dt.int32

_CACHE: dict = {}


def _build_nc(split: bool = True) -> bass.Bass:
    nc = bass.Bass()
    emb_d = nc.dram_tensor("emb", [V, D], BF16, kind="ExternalInput")
    tok_d = nc.dram_tensor("tok", [128, BPC * SBLK], I32, kind="ExternalInput")
    cw_d = nc.dram_tensor("cls_w", [C, D], BF16, kind="ExternalInput")
    cb_d = nc.dram_tensor("cls_b", [C, 1], F32, kind="ExternalInput")
    out_d = nc.dram_tensor("out", [BPC, C], F32, kind="ExternalOutput")

    mult = mybir.AluOpType.mult
    add = mybir.AluOpType.add
    EXP = mybir.ActivationFunctionType.Exp

    with tile.TileContext(nc) as tc:
        with (
            tc.tile_pool(name="const", bufs=1) as constp,
            tc.tile_pool(name="xp", bufs=BPC) as xp,
            tc.tile_pool(name="sp", bufs=2) as sp,
            tc.tile_pool(name="jp", bufs=2) as jp,
            tc.tile_pool(name="tp", bufs=2) as tp,
            tc.tile_pool(name="ps", bufs=2, space="PSUM") as pp,
        ):
            idx = constp.tile([128, BPC, SBLK], I32)
            nc.sync.dma_start(idx[:, :, :], tok_d[:, :])
            ones1 = constp.tile([1, 128], BF16)
            nc.vector.memset(ones1[:], 1.0)
            ones128 = constp.tile([128, 1], F32)
            nc.vector.memset(ones128[:], 1.0)
            cw = constp.tile([C, D], BF16)
            nc.sync.dma_start(cw[:], cw_d[:, :])
            cb = constp.tile([C, 1], F32)
            nc.sync.dma_start(cb[:], cb_d[:, :])
            for b in range(BPC):
                # Gather the 2048 embedding rows for this batch element.
                # Token t lands on partition t%128, free block t//128; one
                # indirect DMA per 128-token block (one index per partition).
                x = xp.tile([128, SBLK, D], BF16, tag="x")
                for j in range(SBLK):
                    nc.gpsimd.indirect_dma_start(
                        out=x[:, j, :],
                        out_offset=None,
                        in_=emb_d[:, :],
                        in_offset=bass.IndirectOffsetOnAxis(
                            ap=idx[:, b, j : j + 1], axis=0
                        ),
                    )

                # Broadcast q = x[token 0] to all 128 partitions via a K=1
                # outer-product matmul: ones[1,128]^T @ x[0:1, 0, :].
                qb = pp.tile([128, D], F32, tag="qb")
                nc.tensor.matmul(qb[:], ones1[:], x[0:1, 0, :], start=True, stop=True)
                qbs = sp.tile([128, D], BF16, tag="qbs")
                nc.scalar.copy(qbs[:], qb[:])

                # Scores s[t] = <x_t, q>: fused multiply+reduce per block
                # ((x*1) * q with accum_out = row sums).
                s = sp.tile([128, SBLK], F32, tag="s")
                for j in range(SBLK):
                    junk = jp.tile([128, D], BF16, tag="junk")
                    nc.vector.scalar_tensor_tensor(
                        out=junk[:],
                        in0=x[:, j, :],
                        scalar=1.0,
                        in1=qbs[:],
                        op0=mult,
                        op1=mult,
                        accum_out=s[:, j : j + 1],
                    )

                # a = exp(s) (scores are O(0.2): no max subtraction needed),
                # with fused per-partition row sums for the softmax denom.
                # Split into groups of 4 blocks so the pooled chain becomes
                # ready incrementally (tracks the gather stream, keeps PE
                # warm, and shrinks the end-of-kernel tail to one group).
                EG = 16
                e = sp.tile([128, SBLK], BF16, tag="e")
                zcols = sp.tile([128, EG], F32, tag="zcols")
                for g in range(EG):
                    lo, hi = g * (SBLK // EG), (g + 1) * (SBLK // EG)
                    nc.scalar.activation(
                        e[:, lo:hi], s[:, lo:hi], EXP,
                        accum_out=zcols[:, g : g + 1],
                    )

                # pooled = a^T X (unnormalized), accumulated over blocks,
                # replicated to C partitions via lhsT free-dim broadcast.
                pooled = pp.tile([C, D], F32, tag="pooled")
                for j in range(SBLK):
                    nc.tensor.matmul(
                        pooled[:],
                        e[:, j : j + 1].broadcast_to([128, C]),
                        x[:, j, :],
                        start=(j == 0),
                        stop=(j == SBLK - 1),
                    )

                # Z = sum over partitions of all zcol groups, C-replicated.
                zps = pp.tile([C, 1], F32, tag="z")
                for g in range(EG):
                    nc.tensor.matmul(
                        zps[:], zcols[:, g : g + 1].broadcast_to([128, C]),
                        ones128[:, :],
                        start=(g == 0), stop=(g == EG - 1),
                    )

                rz = tp.tile([C, 1], F32, tag="rz")
                nc.vector.reciprocal(rz[:], zps[:])

                # logits_c = <pooled, cls_w_c>: one fused dot over C rows,
                # reading pooled directly from PSUM (skips the sbuf copy)
                junk2 = tp.tile([C, D], BF16, tag="junk2")
                lg = tp.tile([C, 1], F32, tag="lg")
                nc.vector.scalar_tensor_tensor(
                    out=junk2[:],
                    in0=pooled[:],
                    scalar=1.0,
                    in1=cw[:],
                    op0=mult,
                    op1=mult,
                    accum_out=lg[:],
                )

                # out = lg / Z + cls_b
                ob = tp.tile([C, 1], F32, tag="ob")
                nc.vector.scalar_tensor_tensor(
                    ob[:], lg[:], rz[:], cb[:], op0=mult, op1=add
                )
                nc.sync.dma_start(out_d[b, :].unsqueeze(1), ob[:, :])

    nc.finalize()
    if split:
        _split_multiwaits(nc)
    return nc


def _wrap_tokens(tokens_row: np.ndarray) -> np.ndarray:
    """[S] int tokens -> [128, SBLK] int32; token t at [t%128, t//128]."""
    return np.ascontiguousarray(tokens_row.reshape(SBLK, 128).T.astype(np.int32))


def get_nc() -> bass.Bass:
    if "nc" not in _CACHE:
        _CACHE["nc"] = _build_nc()
    return _CACHE["nc"]


def make_in_maps(tokens, emb_table, cls_w, cls_b):
    import ml_dtypes

    tokens = np.asarray(tokens)
    emb = np.ascontiguousarray(
        np.asarray(emb_table, dtype=np.float32).astype(ml_dtypes.bfloat16)
    )
    cw = np.ascontiguousarray(
        np.asarray(cls_w, dtype=np.float32).astype(ml_dtypes.bfloat16)
    )
    cb = np.ascontiguousarray(np.asarray(cls_b, dtype=np.float32).reshape(C, 1))
    in_maps = []
    for core in range(N_CORES):
        idx_flat = np.concatenate(
            [_wrap_tokens(tokens[core * BPC + b]) for b in range(BPC)], axis=1
        )  # [128, BPC*SBLK]
        in_maps.append(
            {
                "emb": emb,
                "tok": np.ascontiguousarray(idx_flat),
                "cls_w": cw,
                "cls_b": cb,
            }
        )
    return in_maps


def kernel(tokens, emb_table, cls_w, cls_b) -> np.ndarray:
    nc = get_nc()
    in_maps = make_in_maps(tokens, emb_table, cls_w, cls_b)
    res = run_bass_kernel_spmd(nc, in_maps, core_ids=list(range(N_CORES)))
    outs = [res.results[c]["out"] for c in range(N_CORES)]
    return np.concatenate(outs, axis=0).astype(np.float32)

